# revision 8
# baseline (speedup 1.0000x reference)
"""Grouped-experts MoE (SwiGLU) Bass kernel for Trainium2, 8 NeuronCores.

Expert-parallel: core c owns experts [8c, 8c+8). Tokens are pre-grouped by
expert in the input, so routing is host-side slicing. All device matmuls run
in transposed-token space so every operand streams in its natural layout:

  gateT[i, t] = sum_k G[k, i] * xT[k, t]      (lhsT = G tile, rhs = xT tile)
  hT = silu(gateT) * upT                       (elementwise, [inter, tok])
  outT[m, t] = sum_ki D[ki, m] * hT[ki, t]     (lhsT = D tile, rhs = hT tile)

Host transposes x in / out once per core (not on the device clock).
Compute in bf16 with fp32 PSUM accumulation.
"""

import numpy as np
import ml_dtypes

NUM_EXPERTS = 64
HID = 2048
INTER = 768
N_CORES = 8
EPC = NUM_EXPERTS // N_CORES  # experts per core
KT = HID // 128    # 16 k-tiles over hidden
IT = INTER // 128  # 6 tiles over intermediate
CHUNK = 512        # moving-operand free dim per matmul
SEG_MAX = 1280     # max tokens handled per weight-load segment

BF16_NP = ml_dtypes.bfloat16

_cache = {}


def _chunks(p):
    """Balanced split into ceil(p/CHUNK) near-equal chunks (keeps every
    matmul moving dim wide enough to hide LDWEIGHTS)."""
    if p <= 0:
        return []
    nch = -(-p // CHUNK)
    base, rem = divmod(p, nch)
    out = []
    n0 = 0
    for i in range(nch):
        sz = base + (1 if i < rem else 0)
        out.append((n0, sz))
        n0 += sz
    return out


def _segments(padded):
    """[(slot_idx, col_offset, seg_len)] with seg_len <= SEG_MAX."""
    segs = []
    off = 0
    for j, p in enumerate(padded):
        done = 0
        while done < p:
            take = min(SEG_MAX, p - done)
            segs.append((j, off + done, take))
            done += take
        off += p
    return segs


def _build(padded):
    import concourse.bacc as bacc
    import concourse.mybir as mybir
    import concourse.tile as tile

    BF16 = mybir.dt.bfloat16
    F32 = mybir.dt.float32
    SILU = mybir.ActivationFunctionType.Silu

    ptot = int(sum(padded))
    segs = _segments(padded)
    max_seg = max(s[2] for s in segs)

    nc = bacc.Bacc("TRN2", target_bir_lowering=False, debug=False,
                   num_devices=N_CORES)

    xt = nc.dram_tensor("xt", [HID, ptot], BF16, kind="ExternalInput")
    gw = nc.dram_tensor("gw", [EPC, HID, INTER], BF16, kind="ExternalInput")
    uw = nc.dram_tensor("uw", [EPC, HID, INTER], BF16, kind="ExternalInput")
    dw = nc.dram_tensor("dw", [EPC, INTER, HID], BF16, kind="ExternalInput")
    yt = nc.dram_tensor("yt", [HID, ptot], F32, kind="ExternalOutput")

    # SBUF budget (bytes/partition, 192K usable):
    #   x    17 x max_seg*2   (43.5K at 1280)
    #   w    44 x 1536        (66K)   G/U k-rows [128, INTER]
    #   d     8 x 4096        (32K)
    #   h    20 x 1024        (20K)
    #   silu  4 x 2048        (8K)
    #   out   6 x 2048        (12K)
    bx = max(17, min(24, (48 * 1024) // (max_seg * 2)))

    with tile.TileContext(nc) as tc:
        with (
            tc.tile_pool(name="xp", bufs=bx) as xp,
            tc.tile_pool(name="wp", bufs=44) as wp,
            tc.tile_pool(name="dp", bufs=8) as dp,
            tc.tile_pool(name="hp", bufs=20) as hp,
            tc.tile_pool(name="sp", bufs=4) as sp,
            tc.tile_pool(name="op", bufs=8) as op,
            tc.tile_pool(name="psg", bufs=2, space="PSUM") as psg,
            tc.tile_pool(name="psu", bufs=2, space="PSUM") as psu,
            tc.tile_pool(name="psd", bufs=4, space="PSUM") as psd,
        ):
            for (slot, col0, seg) in segs:
                ch = _chunks(seg)

                # Loads in consumption order on the in-order sync ring:
                # (G_k, x_k) pairs feed the first matmuls ASAP, U next, D last.
                gt, ut, xk = [], [], []
                for k in range(KT):
                    g = wp.tile([128, INTER], BF16, tag="w")
                    nc.sync.dma_start(g[:], gw[slot, k * 128:(k + 1) * 128, :])
                    gt.append(g)
                    t = xp.tile([128, seg], BF16, tag="x")
                    nc.sync.dma_start(t[:], xt[k * 128:(k + 1) * 128,
                                                col0:col0 + seg])
                    xk.append(t)
                for k in range(KT):
                    u = wp.tile([128, INTER], BF16, tag="w")
                    nc.sync.dma_start(u[:], uw[slot, k * 128:(k + 1) * 128, :])
                    ut.append(u)
                dk = []
                for ki in range(IT):
                    d = dp.tile([128, HID], BF16, tag="d")
                    nc.sync.dma_start(d[:], dw[slot, ki * 128:(ki + 1) * 128, :])
                    dk.append(d)

                # ---- gate/up phase ----
                h = {}
                for i in range(IT):
                    for ci, (n0, nsz) in enumerate(ch):
                        pg = psg.tile([128, nsz], F32, tag="pg")
                        for k in range(KT):
                            nc.tensor.matmul(pg[:],
                                             gt[k][:, i * 128:(i + 1) * 128],
                                             xk[k][:, n0:n0 + nsz],
                                             start=(k == 0), stop=(k == KT - 1))
                        pu = psu.tile([128, nsz], F32, tag="pu")
                        for k in range(KT):
                            nc.tensor.matmul(pu[:],
                                             ut[k][:, i * 128:(i + 1) * 128],
                                             xk[k][:, n0:n0 + nsz],
                                             start=(k == 0), stop=(k == KT - 1))
                        st = sp.tile([128, nsz], F32, tag="s")
                        nc.scalar.activation(st[:], pg[:], SILU)
                        ht = hp.tile([128, nsz], BF16, tag="h")
                        nc.vector.tensor_mul(ht[:], st[:], pu[:])
                        h[(i, ci)] = ht

                # ---- down phase ----
                for ci, (n0, nsz) in enumerate(ch):
                    for m in range(KT):
                        pd = psd.tile([128, nsz], F32, tag="pd")
                        for ki in range(IT):
                            nc.tensor.matmul(pd[:],
                                             dk[ki][:, m * 128:(m + 1) * 128],
                                             h[(ki, ci)][:],
                                             start=(ki == 0), stop=(ki == IT - 1))
                        ot = op.tile([128, nsz], F32, tag="o")
                        if m % 2 == 0:
                            nc.scalar.copy(ot[:], pd[:])
                        else:
                            nc.vector.tensor_copy(ot[:], pd[:])
                        nc.gpsimd.dma_start(
                            yt[m * 128:(m + 1) * 128,
                               col0 + n0:col0 + n0 + nsz], ot[:])

    nc.compile()
    return nc, ptot


def _get_program(padded):
    key = tuple(padded)
    if key not in _cache:
        _cache[key] = _build(padded)
    return _cache[key]


def _invoke(x, gate_proj, up_proj, down_proj, num_tokens_per_expert,
            trace=False, trace_kwargs=None):
    from concourse.bass_utils import run_bass_kernel_spmd

    x = np.asarray(x)
    counts = np.asarray(num_tokens_per_expert).astype(np.int64)
    assert counts.shape == (NUM_EXPERTS,)
    starts = np.zeros(NUM_EXPERTS + 1, dtype=np.int64)
    np.cumsum(counts, out=starts[1:])

    # per-slot padded counts (max over cores) -> one SPMD program
    cmat = counts.reshape(N_CORES, EPC)
    padded = [int(cmat[:, j].max()) for j in range(EPC)]
    offs = np.zeros(EPC + 1, dtype=np.int64)
    np.cumsum(np.asarray(padded), out=offs[1:])
    ptot_expected = int(offs[-1])

    nc, ptot = _get_program(padded)
    assert ptot == ptot_expected

    gb = np.asarray(gate_proj).astype(BF16_NP)
    ub = np.asarray(up_proj).astype(BF16_NP)
    db = np.asarray(down_proj).astype(BF16_NP)

    in_maps = []
    for c in range(N_CORES):
        xtc = np.zeros((HID, ptot), dtype=BF16_NP)
        for j in range(EPC):
            e = c * EPC + j
            cnt = int(counts[e])
            if cnt:
                xtc[:, int(offs[j]):int(offs[j]) + cnt] = \
                    x[int(starts[e]):int(starts[e]) + cnt].astype(BF16_NP).T
        in_maps.append({
            "xt": xtc,
            "gw": gb[c * EPC:(c + 1) * EPC],
            "uw": ub[c * EPC:(c + 1) * EPC],
            "dw": db[c * EPC:(c + 1) * EPC],
        })

    res = run_bass_kernel_spmd(nc, in_maps, list(range(N_CORES)),
                               trace=trace, **(trace_kwargs or {}))

    out = np.empty((int(starts[-1]), HID), dtype=np.float32)
    for c in range(N_CORES):
        ytc = res.results[c]["yt"]
        for j in range(EPC):
            e = c * EPC + j
            cnt = int(counts[e])
            if cnt:
                out[int(starts[e]):int(starts[e]) + cnt] = \
                    ytc[:, int(offs[j]):int(offs[j]) + cnt].T
    return out, res


def kernel(x, gate_proj, up_proj, down_proj, num_tokens_per_expert):
    out, _ = _invoke(x, gate_proj, up_proj, down_proj, num_tokens_per_expert)
    return out


# revision 10
# speedup vs baseline: 1.0363x; 1.0363x over previous
"""Grouped-experts MoE (SwiGLU) Bass kernel for Trainium2, 8 NeuronCores.

Expert-parallel: core c owns experts [8c, 8c+8). Tokens are pre-grouped by
expert in the input, so routing is host-side slicing. All device matmuls run
in transposed-token space so every operand streams in its natural layout:

  gateT[i, t] = sum_k G[k, i] * xT[k, t]      (lhsT = G tile, rhs = xT tile)
  hT = silu(gateT) * upT                       (elementwise, [inter, tok])
  outT[m, t] = sum_ki D[ki, m] * hT[ki, t]     (lhsT = D tile, rhs = hT tile)

Host transposes x in / out once per core (not on the device clock).
Compute in bf16 with fp32 PSUM accumulation.
"""

import numpy as np
import ml_dtypes

NUM_EXPERTS = 64
HID = 2048
INTER = 768
N_CORES = 8
EPC = NUM_EXPERTS // N_CORES  # experts per core
KT = HID // 128    # 16 k-tiles over hidden
IT = INTER // 128  # 6 tiles over intermediate
CHUNK = 512        # moving-operand free dim per matmul
SEG_MAX = 1280     # max tokens handled per weight-load segment

BF16_NP = ml_dtypes.bfloat16

_cache = {}


def _chunks(p):
    """Balanced split into ceil(p/CHUNK) near-equal chunks (keeps every
    matmul moving dim wide enough to hide LDWEIGHTS)."""
    if p <= 0:
        return []
    nch = -(-p // CHUNK)
    base, rem = divmod(p, nch)
    out = []
    n0 = 0
    for i in range(nch):
        sz = base + (1 if i < rem else 0)
        out.append((n0, sz))
        n0 += sz
    return out


def _segments(padded):
    """[(slot_idx, col_offset, seg_len)] with seg_len <= SEG_MAX."""
    segs = []
    off = 0
    for j, p in enumerate(padded):
        done = 0
        while done < p:
            take = min(SEG_MAX, p - done)
            segs.append((j, off + done, take))
            done += take
        off += p
    return segs


def _build(padded):
    import concourse.bacc as bacc
    import concourse.mybir as mybir
    import concourse.tile as tile

    BF16 = mybir.dt.bfloat16
    F32 = mybir.dt.float32
    SILU = mybir.ActivationFunctionType.Silu

    ptot = int(sum(padded))
    segs = _segments(padded)
    max_seg = max(s[2] for s in segs)

    nc = bacc.Bacc("TRN2", target_bir_lowering=False, debug=False,
                   num_devices=N_CORES)

    xt = nc.dram_tensor("xt", [HID, ptot], BF16, kind="ExternalInput")
    gw = nc.dram_tensor("gw", [EPC, HID, INTER], BF16, kind="ExternalInput")
    uw = nc.dram_tensor("uw", [EPC, HID, INTER], BF16, kind="ExternalInput")
    dw = nc.dram_tensor("dw", [EPC, INTER, HID], BF16, kind="ExternalInput")
    yt = nc.dram_tensor("yt", [HID, ptot], F32, kind="ExternalOutput")

    # SBUF budget (bytes/partition, 192K usable):
    #   x    17 x max_seg*2   (43.5K at 1280)
    #   w    44 x 1536        (66K)   G/U k-rows [128, INTER]
    #   d     8 x 4096        (32K)
    #   h    20 x 1024        (20K)
    #   silu  4 x 2048        (8K)
    #   out   6 x 2048        (12K)
    bx = max(17, min(24, (44 * 1024) // (max_seg * 2)))

    with tile.TileContext(nc) as tc:
        with (
            tc.tile_pool(name="xp", bufs=bx) as xp,
            tc.tile_pool(name="wp", bufs=44) as wp,
            tc.tile_pool(name="dp", bufs=8) as dp,
            tc.tile_pool(name="hp", bufs=20) as hp,
            tc.tile_pool(name="sp", bufs=4) as sp,
            tc.tile_pool(name="op", bufs=6) as op,
            tc.tile_pool(name="psg", bufs=2, space="PSUM") as psg,
            tc.tile_pool(name="psu", bufs=2, space="PSUM") as psu,
            tc.tile_pool(name="psd", bufs=4, space="PSUM") as psd,
        ):
            for (slot, col0, seg) in segs:
                ch = _chunks(seg)

                # Loads in consumption order on the in-order sync ring:
                # (G_k, x_k) pairs feed the first matmuls ASAP, U next, D last.
                gt, ut, xk = [], [], []
                for k in range(KT):
                    g = wp.tile([128, INTER], BF16, tag="w")
                    nc.sync.dma_start(g[:], gw[slot, k * 128:(k + 1) * 128, :])
                    gt.append(g)
                    t = xp.tile([128, seg], BF16, tag="x")
                    nc.sync.dma_start(t[:], xt[k * 128:(k + 1) * 128,
                                                col0:col0 + seg])
                    xk.append(t)
                for k in range(KT):
                    u = wp.tile([128, INTER], BF16, tag="w")
                    nc.sync.dma_start(u[:], uw[slot, k * 128:(k + 1) * 128, :])
                    ut.append(u)
                dk = []
                for ki in range(IT):
                    d = dp.tile([128, HID], BF16, tag="d")
                    nc.sync.dma_start(d[:], dw[slot, ki * 128:(ki + 1) * 128, :])
                    dk.append(d)

                # ---- gate/up phase ----
                h = {}
                for i in range(IT):
                    for ci, (n0, nsz) in enumerate(ch):
                        pg = psg.tile([128, nsz], F32, tag="pg")
                        for k in range(KT):
                            nc.tensor.matmul(pg[:],
                                             gt[k][:, i * 128:(i + 1) * 128],
                                             xk[k][:, n0:n0 + nsz],
                                             start=(k == 0), stop=(k == KT - 1))
                        pu = psu.tile([128, nsz], F32, tag="pu")
                        for k in range(KT):
                            nc.tensor.matmul(pu[:],
                                             ut[k][:, i * 128:(i + 1) * 128],
                                             xk[k][:, n0:n0 + nsz],
                                             start=(k == 0), stop=(k == KT - 1))
                        st = sp.tile([128, nsz], F32, tag="s")
                        nc.scalar.activation(st[:], pg[:], SILU)
                        ht = hp.tile([128, nsz], BF16, tag="h")
                        nc.vector.tensor_mul(ht[:], st[:], pu[:])
                        h[(i, ci)] = ht

                # ---- down phase ----
                for ci, (n0, nsz) in enumerate(ch):
                    for m in range(KT):
                        pd = psd.tile([128, nsz], F32, tag="pd")
                        for ki in range(IT):
                            nc.tensor.matmul(pd[:],
                                             dk[ki][:, m * 128:(m + 1) * 128],
                                             h[(ki, ci)][:],
                                             start=(ki == 0), stop=(ki == IT - 1))
                        ot = op.tile([128, nsz], F32, tag="o")
                        if m % 2 == 0:
                            nc.scalar.copy(ot[:], pd[:])
                        else:
                            nc.vector.tensor_copy(ot[:], pd[:])
                        nc.gpsimd.dma_start(
                            yt[m * 128:(m + 1) * 128,
                               col0 + n0:col0 + n0 + nsz], ot[:])

    nc.compile()
    return nc, ptot


def _get_program(padded):
    key = tuple(padded)
    if key not in _cache:
        _cache[key] = _build(padded)
    return _cache[key]


def _invoke(x, gate_proj, up_proj, down_proj, num_tokens_per_expert,
            trace=False, trace_kwargs=None):
    from concourse.bass_utils import run_bass_kernel_spmd

    x = np.asarray(x)
    counts = np.asarray(num_tokens_per_expert).astype(np.int64)
    assert counts.shape == (NUM_EXPERTS,)
    starts = np.zeros(NUM_EXPERTS + 1, dtype=np.int64)
    np.cumsum(counts, out=starts[1:])

    # per-slot padded counts (max over cores) -> one SPMD program
    cmat = counts.reshape(N_CORES, EPC)
    padded = [int(cmat[:, j].max()) for j in range(EPC)]
    offs = np.zeros(EPC + 1, dtype=np.int64)
    np.cumsum(np.asarray(padded), out=offs[1:])
    ptot_expected = int(offs[-1])

    nc, ptot = _get_program(padded)
    assert ptot == ptot_expected

    gb = np.asarray(gate_proj).astype(BF16_NP)
    ub = np.asarray(up_proj).astype(BF16_NP)
    db = np.asarray(down_proj).astype(BF16_NP)

    in_maps = []
    for c in range(N_CORES):
        xtc = np.zeros((HID, ptot), dtype=BF16_NP)
        for j in range(EPC):
            e = c * EPC + j
            cnt = int(counts[e])
            if cnt:
                xtc[:, int(offs[j]):int(offs[j]) + cnt] = \
                    x[int(starts[e]):int(starts[e]) + cnt].astype(BF16_NP).T
        in_maps.append({
            "xt": xtc,
            "gw": gb[c * EPC:(c + 1) * EPC],
            "uw": ub[c * EPC:(c + 1) * EPC],
            "dw": db[c * EPC:(c + 1) * EPC],
        })

    res = run_bass_kernel_spmd(nc, in_maps, list(range(N_CORES)),
                               trace=trace, **(trace_kwargs or {}))

    out = np.empty((int(starts[-1]), HID), dtype=np.float32)
    for c in range(N_CORES):
        ytc = res.results[c]["yt"]
        for j in range(EPC):
            e = c * EPC + j
            cnt = int(counts[e])
            if cnt:
                out[int(starts[e]):int(starts[e]) + cnt] = \
                    ytc[:, int(offs[j]):int(offs[j]) + cnt].T
    return out, res


def kernel(x, gate_proj, up_proj, down_proj, num_tokens_per_expert):
    out, _ = _invoke(x, gate_proj, up_proj, down_proj, num_tokens_per_expert)
    return out
